# revision 6
# baseline (speedup 1.0000x reference)
"""Trainium2 Bass kernel for nn_ContrastiveLoss (B=65536, O=4, H=1024).

Strategy (pure data parallel over 8 NeuronCores):
  - Each core gets 8192 samples ([8192, 4096] f32 = 128 MiB) and streams them
    through SBUF in 64 tiles of [128 samples, 4096].
  - Per tile, per sample: 4 squared norms (ACT Square + accum_out) and 6
    cross-pair dot products (DVE tensor_tensor_reduce), accumulated into
    persistent [128, k, 64] SBUF columns.
  - Phase 2 (once per core, tiny [128, 64]-shaped math): cosine sims via
    exp(-0.5*ln(sq_i*sq_j)), exp(sim/T), label masks, pos/neg loss terms,
    per-partition (loss_sum, count) reduced to a [128, 2] output.
  - Host: sum the 8 cores' [128, 2] partials, divide.
"""

import numpy as np

TEMPERATURE = 0.07
B, O, H = 65536, 4, 1024
N_CORES = 8
B_SHARD = B // N_CORES          # 8192
P = 128
N_TILES = B_SHARD // P          # 64
PAIRS = [(0, 1), (0, 2), (0, 3), (1, 2), (1, 3), (2, 3)]

_CACHE = {}

_MAX_WAITS = 1  # this walrus build encodes at most 1 sync-wait per instruction


def _split_waits_json_bytes(bir_json: bytes) -> bytes:
    """Split instructions carrying >_MAX_WAITS sem-waits into NoOp prefixes.

    Tile attaches multiple semaphore waits to one instruction (epilogue drain
    especially); this compiler rejects that. Same-engine program order makes
    hoisting excess waits onto preceding NoOps semantically identical.
    """
    import json

    bir = json.loads(bir_json)
    for fn in bir.get("functions", []):
        for bb in fn.get("blocks", []):
            out = []
            for ins in bb.get("instructions", []):
                si = ins.get("sync_info")
                waits = (si or {}).get("on_wait") or []
                if len(waits) > _MAX_WAITS:
                    chunks = [
                        waits[i : i + _MAX_WAITS]
                        for i in range(0, len(waits), _MAX_WAITS)
                    ]
                    for ci, chunk in enumerate(chunks[:-1]):
                        nop = {
                            "name": f"{ins['name']}_w{ci}",
                            "opcode": "NoOp",
                            "engine": ins["engine"],
                            "ins": [],
                            "outs": [],
                            "sync_info": {"on_update": [], "on_wait": chunk},
                        }
                        if "debug" in ins:
                            nop["debug"] = ins["debug"]
                        out.append(nop)
                    si["on_wait"] = chunks[-1]
                out.append(ins)
            bb["instructions"] = out
    return json.dumps(bir).encode()


def _patch_bass(nc):
    import concourse.mybir as mybir

    def to_json_bytes():
        return _split_waits_json_bytes(mybir.module_to_json_bytes(nc.m))

    nc.to_json_bytes = to_json_bytes
    return nc


def build(n_tiles=N_TILES):
    import concourse.bass as bass
    import concourse.mybir as mybir
    import concourse.tile as tile

    f32 = mybir.dt.float32
    A = mybir.AluOpType
    F = mybir.ActivationFunctionType

    nc = bass.Bass()
    emb = nc.dram_tensor("emb", [n_tiles * P, O * H], f32, kind="ExternalInput")
    pmask = nc.dram_tensor("pmask", [P, O * n_tiles], f32, kind="ExternalInput")
    out = nc.dram_tensor("out", [P, 2], f32, kind="ExternalOutput")

    emb_t = emb.rearrange("(t p) f -> t p f", p=P)
    pmask_v = pmask.rearrange("p (o t) -> p o t", o=O)

    with tile.TileContext(nc) as tc:
        with (
            tc.tile_pool(name="io", bufs=4) as io,
            tc.tile_pool(name="scr_a", bufs=2) as scr_a,
            tc.tile_pool(name="scr_d", bufs=2) as scr_d,
            tc.tile_pool(name="acc", bufs=1) as accp,
            tc.tile_pool(name="ph2", bufs=1) as ph2,
        ):
            SQ = accp.tile([P, O, n_tiles], f32, tag="SQ", name="SQ")
            D = accp.tile([P, 6, n_tiles], f32, tag="D", name="D")
            PM = accp.tile([P, O, n_tiles], f32, tag="PM", name="PM")
            nc.sync.dma_start(out=PM, in_=pmask_v)

            # ---- Phase 1: stream embeddings, accumulate dots ----
            for t in range(n_tiles):
                et = io.tile([P, O * H], f32, tag="et", name="et")
                nc.sync.dma_start(out=et, in_=emb_t[t])
                for i in range(O):
                    sa = scr_a.tile([P, H], f32, tag="sa", name="sa")
                    nc.scalar.activation(
                        out=sa,
                        in_=et[:, i * H : (i + 1) * H],
                        func=F.Square,
                        accum_out=SQ[:, i, t : t + 1],
                    )
                for k, (i, j) in enumerate(PAIRS):
                    sd = scr_d.tile([P, H], f32, tag="sd", name="sd")
                    nc.vector.scalar_tensor_tensor(
                        out=sd,
                        in0=et[:, i * H : (i + 1) * H],
                        scalar=1.0,
                        in1=et[:, j * H : (j + 1) * H],
                        op0=A.mult,
                        op1=A.mult,
                        accum_out=D[:, k, t : t + 1],
                    )

            # ---- Phase 2: per-sample scalar math on [P, n_tiles] columns ----
            n = n_tiles

            def t6(tag):
                return ph2.tile([P, 6, n], f32, tag=tag, name=tag)

            def t4(tag):
                return ph2.tile([P, O, n], f32, tag=tag, name=tag)

            def t1(tag):
                return ph2.tile([P, n], f32, tag=tag, name=tag)

            V = nc.vector
            S = nc.scalar

            # cross-pair cosine sims and exp
            SQI, SQJ = t6("SQI"), t6("SQJ")
            for k, (i, j) in enumerate(PAIRS):
                V.tensor_copy(out=SQI[:, k, :], in_=SQ[:, i, :])
                V.tensor_copy(out=SQJ[:, k, :], in_=SQ[:, j, :])
            M = t6("M")
            V.tensor_tensor(out=M, in0=SQI, in1=SQJ, op=A.mult)
            LNM = t6("LNM")
            S.activation(out=LNM, in_=M, func=F.Ln)
            W = t6("W")
            S.activation(out=W, in_=LNM, func=F.Exp, scale=-0.5)  # rsqrt(M)
            COS = t6("COS")
            V.tensor_tensor(out=COS, in0=D, in1=W, op=A.mult)
            ES = t6("ES")
            S.activation(out=ES, in_=COS, func=F.Exp, scale=1.0 / TEMPERATURE)

            # diagonal terms: cos_ii = sq * (1/sq)
            LNS = t4("LNS")
            S.activation(out=LNS, in_=SQ, func=F.Ln)
            WD = t4("WD")
            S.activation(out=WD, in_=LNS, func=F.Exp, scale=-1.0)  # 1/sq
            CD = t4("CD")
            V.tensor_tensor(out=CD, in0=SQ, in1=WD, op=A.mult)
            ESD = t4("ESD")
            S.activation(out=ESD, in_=CD, func=F.Exp, scale=1.0 / TEMPERATURE)

            # label masks
            Pn = t1("Pn")
            V.tensor_tensor(out=Pn, in0=PM[:, 0, :], in1=PM[:, 1, :], op=A.add)
            V.tensor_tensor(out=Pn, in0=Pn, in1=PM[:, 2, :], op=A.add)
            V.tensor_tensor(out=Pn, in0=Pn, in1=PM[:, 3, :], op=A.add)
            U = t1("U")  # (P > 1) indicator
            V.tensor_scalar(out=U, in0=Pn, scalar1=1.0, scalar2=0.0, op0=A.subtract, op1=A.max)
            V.tensor_scalar(out=U, in0=U, scalar1=1.0, scalar2=None, op0=A.min)
            V4 = t1("V4")  # 4 - P
            V.tensor_scalar(out=V4, in0=Pn, scalar1=-1.0, scalar2=4.0, op0=A.mult, op1=A.add)
            VAL = t1("VAL")  # valid = (P>0)&(P<4)
            V.tensor_tensor(out=VAL, in0=Pn, in1=V4, op=A.mult)
            V.tensor_scalar(out=VAL, in0=VAL, scalar1=1.0, scalar2=None, op0=A.min)
            HPP = t1("HPP")  # has_pos_pair = valid & (P>1)
            V.tensor_tensor(out=HPP, in0=VAL, in1=U, op=A.mult)
            CNT = t1("CNT")  # count = valid*P + hpp
            V.tensor_tensor(out=CNT, in0=VAL, in1=Pn, op=A.mult)
            V.tensor_tensor(out=CNT, in0=CNT, in1=HPP, op=A.add)

            # positive-pair mean term
            PI, PJ = t6("PI"), t6("PJ")
            for k, (i, j) in enumerate(PAIRS):
                V.tensor_copy(out=PI[:, k, :], in_=PM[:, i, :])
                V.tensor_copy(out=PJ[:, k, :], in_=PM[:, j, :])
            PPRO = t6("PPRO")
            V.tensor_tensor(out=PPRO, in0=PI, in1=PJ, op=A.mult)
            EPP = t6("EPP")
            V.tensor_tensor(out=EPP, in0=ES, in1=PPRO, op=A.mult)
            CS = t1("CS")
            V.tensor_tensor(out=CS, in0=EPP[:, 0, :], in1=EPP[:, 1, :], op=A.add)
            for k in range(2, 6):
                V.tensor_tensor(out=CS, in0=CS, in1=EPP[:, k, :], op=A.add)
            EPD = t4("EPD")
            V.tensor_tensor(out=EPD, in0=ESD, in1=PM, op=A.mult)
            DS = t1("DS")
            V.tensor_tensor(out=DS, in0=EPD[:, 0, :], in1=EPD[:, 1, :], op=A.add)
            V.tensor_tensor(out=DS, in0=DS, in1=EPD[:, 2, :], op=A.add)
            V.tensor_tensor(out=DS, in0=DS, in1=EPD[:, 3, :], op=A.add)
            PS = t1("PS")  # pairsum = 2*cross + diag
            V.scalar_tensor_tensor(out=PS, in0=CS, scalar=2.0, in1=DS, op0=A.mult, op1=A.add)
            P2 = t1("P2")
            V.tensor_tensor(out=P2, in0=Pn, in1=Pn, op=A.mult)
            V.tensor_scalar(out=P2, in0=P2, scalar1=1.0, scalar2=None, op0=A.max)
            IDEN = t1("IDEN")
            V.reciprocal(out=IDEN, in_=P2)
            PMEAN = t1("PMEAN")
            V.tensor_tensor(out=PMEAN, in0=PS, in1=IDEN, op=A.mult)
            V.tensor_scalar(out=PMEAN, in0=PMEAN, scalar1=1e-30, scalar2=None, op0=A.max)
            LG = t1("LG")
            S.activation(out=LG, in_=PMEAN, func=F.Ln)
            POSL = t1("POSL")
            V.tensor_tensor(out=POSL, in0=LG, in1=HPP, op=A.mult)

            # negative term: per positive row i, log1p(sum_j esim_ij * neg_j)
            NB = t4("NB")
            V.tensor_scalar(out=NB, in0=PM, scalar1=-1.0, scalar2=1.0, op0=A.mult, op1=A.add)
            pair_idx = {}
            for k, (i, j) in enumerate(PAIRS):
                pair_idx[(i, j)] = k
                pair_idx[(j, i)] = k
            NEGACC = t1("NEGACC")
            for i in range(O):
                NS = t1(f"NS{i}")
                V.tensor_tensor(out=NS, in0=ESD[:, i, :], in1=NB[:, i, :], op=A.mult)
                TMP = t1(f"TMP{i}")
                for j in range(O):
                    if j == i:
                        continue
                    k = pair_idx[(i, j)]
                    V.tensor_tensor(out=TMP, in0=ES[:, k, :], in1=NB[:, j, :], op=A.mult)
                    V.tensor_tensor(out=NS, in0=NS, in1=TMP, op=A.add)
                V.tensor_scalar(out=NS, in0=NS, scalar1=1.0, scalar2=None, op0=A.add)
                S.activation(out=NS, in_=NS, func=F.Ln)
                V.tensor_tensor(out=NS, in0=NS, in1=PM[:, i, :], op=A.mult)
                if i == 0:
                    V.tensor_copy(out=NEGACC, in_=NS)
                else:
                    V.tensor_tensor(out=NEGACC, in0=NEGACC, in1=NS, op=A.add)
            NEGL = t1("NEGL")
            V.tensor_tensor(out=NEGL, in0=NEGACC, in1=VAL, op=A.mult)

            LOSS = t1("LOSS")
            V.tensor_tensor(out=LOSS, in0=NEGL, in1=POSL, op=A.subtract)

            OUTSB = ph2.tile([P, 2], f32, tag="OUTSB", name="OUTSB")
            V.tensor_reduce(out=OUTSB[:, 0:1], in_=LOSS, axis=mybir.AxisListType.X, op=A.add)
            V.tensor_reduce(out=OUTSB[:, 1:2], in_=CNT, axis=mybir.AxisListType.X, op=A.add)
            nc.sync.dma_start(out=out[:, :], in_=OUTSB)

    return _patch_bass(nc)


def _prep_inputs(embeddings, labels):
    emb = np.ascontiguousarray(np.asarray(embeddings, dtype=np.float32)).reshape(B, O * H)
    posf = (np.asarray(labels) == 1).astype(np.float32)  # [B, O]
    in_maps = []
    for c in range(N_CORES):
        sl = slice(c * B_SHARD, (c + 1) * B_SHARD)
        pm = posf[sl].reshape(N_TILES, P, O).transpose(1, 2, 0).reshape(P, O * N_TILES)
        in_maps.append(
            {
                "emb": np.ascontiguousarray(emb[sl]),
                "pmask": np.ascontiguousarray(pm),
            }
        )
    return in_maps


def kernel(embeddings, labels):
    from concourse.bass_utils import run_bass_kernel_spmd

    if "nc" not in _CACHE:
        _CACHE["nc"] = build()
    nc = _CACHE["nc"]

    in_maps = _prep_inputs(embeddings, labels)
    res = run_bass_kernel_spmd(nc, in_maps, core_ids=list(range(N_CORES)))
    loss_sum = 0.0
    count_sum = 0.0
    for r in res.results:
        o = r["out"].astype(np.float64)
        loss_sum += o[:, 0].sum()
        count_sum += o[:, 1].sum()
    return np.float32(loss_sum / max(count_sum, 1.0))


# revision 8
# speedup vs baseline: 1.0197x; 1.0197x over previous
"""Trainium2 Bass kernel for nn_ContrastiveLoss (B=65536, O=4, H=1024).

Strategy (pure data parallel over 8 NeuronCores):
  - Each core gets 8192 samples (128 MiB f32) and streams them through SBUF
    in 32 groups of [128 partitions, 2 sample-tiles x 4096], cast to bf16
    during the DMA (memory traffic stays f32-sized on the HBM side).
  - Per sample-tile of 128 samples, 10 free-dim folds produce the per-sample
    Gram entries: 5 cross-pair dot products on DVE (scalar_tensor_tensor with
    accum_out), 4 squared norms + 1 sum-square (pair (0,1) via the
    polarization identity) on ACT (activation Square with accum_out); the
    (E0+E1) sum is a cheap 2x-mode bf16 tensor_tensor add on DVE. This
    balances DVE ~7.2us vs ACT ~6.9us per tile.
  - Phase 2 (tiny [128, 16]-column math, 4 blocks overlapped with phase 1):
    cosine sims via exp(-0.5*ln(sq_i*sq_j)), exp(sim/T), label masks,
    pos/neg loss terms, per-partition (loss_sum, count) into [128, 2].
  - Host: sum the 8 cores' [128, 2] partials, divide.
"""

import numpy as np

TEMPERATURE = 0.07
B, O, H = 65536, 4, 1024
N_CORES = 8
B_SHARD = B // N_CORES          # 8192
P = 128
N_TILES = B_SHARD // P          # 64
GROUP = 2                       # sample-tiles per DMA
N_BLOCKS = 4                    # phase-2 column blocks (overlap with phase 1)
# pair 0 is computed via the ACT sum-square route; 1..5 directly on DVE
PAIRS = [(0, 1), (0, 2), (0, 3), (1, 2), (1, 3), (2, 3)]
COMPUTE_BF16 = True

_CACHE = {}

_MAX_WAITS = 1  # this walrus build encodes at most 1 sync-wait per instruction


def _split_waits_json_bytes(bir_json: bytes) -> bytes:
    """Split instructions carrying >_MAX_WAITS sem-waits into NoOp prefixes.

    Tile attaches multiple semaphore waits to one instruction (epilogue drain
    especially); this compiler rejects that. Same-engine program order makes
    hoisting excess waits onto preceding NoOps semantically identical.
    """
    import json

    bir = json.loads(bir_json)
    for fn in bir.get("functions", []):
        for bb in fn.get("blocks", []):
            out = []
            for ins in bb.get("instructions", []):
                si = ins.get("sync_info")
                waits = (si or {}).get("on_wait") or []
                if len(waits) > _MAX_WAITS:
                    chunks = [
                        waits[i : i + _MAX_WAITS]
                        for i in range(0, len(waits), _MAX_WAITS)
                    ]
                    for ci, chunk in enumerate(chunks[:-1]):
                        nop = {
                            "name": f"{ins['name']}_w{ci}",
                            "opcode": "NoOp",
                            "engine": ins["engine"],
                            "ins": [],
                            "outs": [],
                            "sync_info": {"on_update": [], "on_wait": chunk},
                        }
                        if "debug" in ins:
                            nop["debug"] = ins["debug"]
                        out.append(nop)
                    si["on_wait"] = chunks[-1]
                out.append(ins)
            bb["instructions"] = out
    return json.dumps(bir).encode()


def _patch_bass(nc):
    import concourse.mybir as mybir

    def to_json_bytes():
        return _split_waits_json_bytes(mybir.module_to_json_bytes(nc.m))

    nc.to_json_bytes = to_json_bytes
    return nc


def build(n_tiles=N_TILES):
    import concourse.bass as bass
    import concourse.mybir as mybir
    import concourse.tile as tile

    f32 = mybir.dt.float32
    cdt = mybir.dt.bfloat16 if COMPUTE_BF16 else f32
    A = mybir.AluOpType
    F = mybir.ActivationFunctionType

    nc = bass.Bass()
    emb = nc.dram_tensor("emb", [n_tiles * P, O * H], f32, kind="ExternalInput")
    pmask = nc.dram_tensor("pmask", [P, O * n_tiles], f32, kind="ExternalInput")
    out = nc.dram_tensor("out", [P, 2], f32, kind="ExternalOutput")

    group = GROUP if n_tiles % GROUP == 0 else 1
    n_groups = n_tiles // group
    emb_g = emb.rearrange("(g u p) f -> g p u f", p=P, u=group)
    pmask_v = pmask.rearrange("p (o t) -> p o t", o=O)

    n_blocks = N_BLOCKS if n_tiles % N_BLOCKS == 0 else 1
    blk = n_tiles // n_blocks

    with tile.TileContext(nc) as tc:
        with (
            tc.tile_pool(name="io", bufs=3) as io,
            tc.tile_pool(name="scr_a", bufs=2) as scr_a,
            tc.tile_pool(name="scr_d", bufs=2) as scr_d,
            tc.tile_pool(name="sum01", bufs=2) as sum01p,
            tc.tile_pool(name="acc", bufs=1) as accp,
            tc.tile_pool(name="ph2", bufs=1) as ph2,
        ):
            SQ = accp.tile([P, O, n_tiles], f32, tag="SQ", name="SQ")
            # D slot 0 holds the raw sum-square SS_01; slots 1..5 direct dots
            D = accp.tile([P, 6, n_tiles], f32, tag="D", name="D")
            PM = accp.tile([P, O, n_tiles], f32, tag="PM", name="PM")
            nc.sync.dma_start(out=PM, in_=pmask_v)

            V = nc.vector
            S = nc.scalar

            def phase1_tile(sl, t):
                # sl: [P, O*H] cdt slice for sample-tile t
                ei = [sl[:, i * H : (i + 1) * H] for i in range(O)]
                # ACT: 4 squared norms
                for i in range(O):
                    sa = scr_a.tile([P, H], cdt, tag="sa", name="sa")
                    S.activation(
                        out=sa, in_=ei[i], func=F.Square,
                        accum_out=SQ[:, i, t : t + 1],
                    )
                # DVE: sum for the ACT-route pair (0,1), then ACT sum-square
                s01 = sum01p.tile([P, H], cdt, tag="s01", name="s01")
                V.tensor_tensor(out=s01, in0=ei[0], in1=ei[1], op=A.add)
                sa = scr_a.tile([P, H], cdt, tag="sa", name="sa")
                S.activation(
                    out=sa, in_=s01, func=F.Square,
                    accum_out=D[:, 0, t : t + 1],
                )
                # DVE: 5 direct cross-pair dots
                for k, (i, j) in enumerate(PAIRS[1:], start=1):
                    sd = scr_d.tile([P, H], cdt, tag="sd", name="sd")
                    V.scalar_tensor_tensor(
                        out=sd, in0=ei[i], scalar=1.0, in1=ei[j],
                        op0=A.mult, op1=A.mult,
                        accum_out=D[:, k, t : t + 1],
                    )

            def t6(tag, b):
                return ph2.tile([P, 6, blk], f32, tag=tag, name=tag)

            def t4(tag, b):
                return ph2.tile([P, O, blk], f32, tag=tag, name=tag)

            def t1(tag, b):
                return ph2.tile([P, blk], f32, tag=tag, name=tag)

            LOSS_C = ph2.tile([P, n_tiles], f32, tag="LOSS_C", name="LOSS_C")
            CNT_C = ph2.tile([P, n_tiles], f32, tag="CNT_C", name="CNT_C")

            def phase2_block(b):
                c0, c1 = b * blk, (b + 1) * blk
                SQb = SQ[:, :, c0:c1]
                Db = D[:, :, c0:c1]
                PMb = PM[:, :, c0:c1]

                # finish pair 0: D_01 = (SS - sq0 - sq1) / 2
                D01 = t1("D01", b)
                V.tensor_tensor(out=D01, in0=Db[:, 0, :], in1=SQb[:, 0, :], op=A.subtract)
                V.tensor_tensor(out=D01, in0=D01, in1=SQb[:, 1, :], op=A.subtract)

                # cross-pair cosine sims and exp
                SQI, SQJ = t6("SQI", b), t6("SQJ", b)
                for k, (i, j) in enumerate(PAIRS):
                    V.tensor_copy(out=SQI[:, k, :], in_=SQb[:, i, :])
                    V.tensor_copy(out=SQJ[:, k, :], in_=SQb[:, j, :])
                M = t6("M", b)
                V.tensor_tensor(out=M, in0=SQI, in1=SQJ, op=A.mult)
                W = t6("W", b)
                S.activation(out=W, in_=M, func=F.Ln)
                S.activation(out=W, in_=W, func=F.Exp, scale=-0.5)  # rsqrt(M)
                COS = t6("COS", b)
                V.tensor_tensor(out=COS[:, 0, :], in0=D01, in1=W[:, 0, :], op=A.mult)
                V.tensor_scalar(out=COS[:, 0, :], in0=COS[:, 0, :], scalar1=0.5,
                                scalar2=None, op0=A.mult)
                V.tensor_tensor(out=COS[:, 1:6, :], in0=Db[:, 1:6, :],
                                in1=W[:, 1:6, :], op=A.mult)
                ES = t6("ES", b)
                S.activation(out=ES, in_=COS, func=F.Exp, scale=1.0 / TEMPERATURE)

                # diagonal terms: cos_ii = sq * (1/sq)
                WD = t4("WD", b)
                S.activation(out=WD, in_=SQb, func=F.Ln)
                S.activation(out=WD, in_=WD, func=F.Exp, scale=-1.0)  # 1/sq
                CD = t4("CD", b)
                V.tensor_tensor(out=CD, in0=SQb, in1=WD, op=A.mult)
                ESD = t4("ESD", b)
                S.activation(out=ESD, in_=CD, func=F.Exp, scale=1.0 / TEMPERATURE)

                # label masks
                Pn = t1("Pn", b)
                V.tensor_tensor(out=Pn, in0=PMb[:, 0, :], in1=PMb[:, 1, :], op=A.add)
                V.tensor_tensor(out=Pn, in0=Pn, in1=PMb[:, 2, :], op=A.add)
                V.tensor_tensor(out=Pn, in0=Pn, in1=PMb[:, 3, :], op=A.add)
                U = t1("U", b)  # (P > 1) indicator
                V.tensor_scalar(out=U, in0=Pn, scalar1=1.0, scalar2=0.0,
                                op0=A.subtract, op1=A.max)
                V.tensor_scalar(out=U, in0=U, scalar1=1.0, scalar2=None, op0=A.min)
                V4 = t1("V4", b)  # 4 - P
                V.tensor_scalar(out=V4, in0=Pn, scalar1=-1.0, scalar2=4.0,
                                op0=A.mult, op1=A.add)
                VAL = t1("VAL", b)  # valid = (P>0)&(P<4)
                V.tensor_tensor(out=VAL, in0=Pn, in1=V4, op=A.mult)
                V.tensor_scalar(out=VAL, in0=VAL, scalar1=1.0, scalar2=None, op0=A.min)
                HPP = t1("HPP", b)  # has_pos_pair = valid & (P>1)
                V.tensor_tensor(out=HPP, in0=VAL, in1=U, op=A.mult)
                CNT = CNT_C[:, c0:c1]  # count = valid*P + hpp
                V.tensor_tensor(out=CNT, in0=VAL, in1=Pn, op=A.mult)
                V.tensor_tensor(out=CNT, in0=CNT, in1=HPP, op=A.add)

                # positive-pair mean term
                PI, PJ = t6("PI", b), t6("PJ", b)
                for k, (i, j) in enumerate(PAIRS):
                    V.tensor_copy(out=PI[:, k, :], in_=PMb[:, i, :])
                    V.tensor_copy(out=PJ[:, k, :], in_=PMb[:, j, :])
                PPRO = t6("PPRO", b)
                V.tensor_tensor(out=PPRO, in0=PI, in1=PJ, op=A.mult)
                EPP = t6("EPP", b)
                V.tensor_tensor(out=EPP, in0=ES, in1=PPRO, op=A.mult)
                CS = t1("CS", b)
                V.tensor_tensor(out=CS, in0=EPP[:, 0, :], in1=EPP[:, 1, :], op=A.add)
                for k in range(2, 6):
                    V.tensor_tensor(out=CS, in0=CS, in1=EPP[:, k, :], op=A.add)
                EPD = t4("EPD", b)
                V.tensor_tensor(out=EPD, in0=ESD, in1=PMb, op=A.mult)
                DS = t1("DS", b)
                V.tensor_tensor(out=DS, in0=EPD[:, 0, :], in1=EPD[:, 1, :], op=A.add)
                V.tensor_tensor(out=DS, in0=DS, in1=EPD[:, 2, :], op=A.add)
                V.tensor_tensor(out=DS, in0=DS, in1=EPD[:, 3, :], op=A.add)
                PS = t1("PS", b)  # pairsum = 2*cross + diag
                V.scalar_tensor_tensor(out=PS, in0=CS, scalar=2.0, in1=DS,
                                       op0=A.mult, op1=A.add)
                P2 = t1("P2", b)
                V.tensor_tensor(out=P2, in0=Pn, in1=Pn, op=A.mult)
                V.tensor_scalar(out=P2, in0=P2, scalar1=1.0, scalar2=None, op0=A.max)
                IDEN = t1("IDEN", b)
                V.reciprocal(out=IDEN, in_=P2)
                PMEAN = t1("PMEAN", b)
                V.tensor_tensor(out=PMEAN, in0=PS, in1=IDEN, op=A.mult)
                V.tensor_scalar(out=PMEAN, in0=PMEAN, scalar1=1e-30, scalar2=None,
                                op0=A.max)
                LG = t1("LG", b)
                S.activation(out=LG, in_=PMEAN, func=F.Ln)
                POSL = t1("POSL", b)
                V.tensor_tensor(out=POSL, in0=LG, in1=HPP, op=A.mult)

                # negative term: per positive row i, log1p(sum_j esim_ij*neg_j)
                NB = t4("NB", b)
                V.tensor_scalar(out=NB, in0=PMb, scalar1=-1.0, scalar2=1.0,
                                op0=A.mult, op1=A.add)
                pair_idx = {}
                for k, (i, j) in enumerate(PAIRS):
                    pair_idx[(i, j)] = k
                    pair_idx[(j, i)] = k
                NEGACC = t1("NEGACC", b)
                for i in range(O):
                    NS = t1(f"NS{i}", b)
                    V.tensor_tensor(out=NS, in0=ESD[:, i, :], in1=NB[:, i, :], op=A.mult)
                    TMP = t1(f"TMP{i}", b)
                    for j in range(O):
                        if j == i:
                            continue
                        k = pair_idx[(i, j)]
                        V.tensor_tensor(out=TMP, in0=ES[:, k, :], in1=NB[:, j, :],
                                        op=A.mult)
                        V.tensor_tensor(out=NS, in0=NS, in1=TMP, op=A.add)
                    V.tensor_scalar(out=NS, in0=NS, scalar1=1.0, scalar2=None, op0=A.add)
                    S.activation(out=NS, in_=NS, func=F.Ln)
                    V.tensor_tensor(out=NS, in0=NS, in1=PMb[:, i, :], op=A.mult)
                    if i == 0:
                        V.tensor_copy(out=NEGACC, in_=NS)
                    else:
                        V.tensor_tensor(out=NEGACC, in0=NEGACC, in1=NS, op=A.add)
                NEGL = t1("NEGL", b)
                V.tensor_tensor(out=NEGL, in0=NEGACC, in1=VAL, op=A.mult)

                V.tensor_tensor(out=LOSS_C[:, c0:c1], in0=NEGL, in1=POSL,
                                op=A.subtract)

            # ---- Phase 1 + interleaved phase-2 blocks ----
            done_blocks = 0
            for g in range(n_groups):
                et = io.tile([P, group, O * H], cdt, tag="et", name="et")
                if COMPUTE_BF16:
                    nc.gpsimd.dma_start(out=et, in_=emb_g[g])  # f32 -> bf16 cast
                else:
                    nc.sync.dma_start(out=et, in_=emb_g[g])
                for u in range(group):
                    t = g * group + u
                    phase1_tile(et[:, u, :], t)
                while (done_blocks + 1) * blk <= (g + 1) * group and done_blocks < n_blocks:
                    phase2_block(done_blocks)
                    done_blocks += 1
            while done_blocks < n_blocks:
                phase2_block(done_blocks)
                done_blocks += 1

            OUTSB = ph2.tile([P, 2], f32, tag="OUTSB", name="OUTSB")
            V.tensor_reduce(out=OUTSB[:, 0:1], in_=LOSS_C,
                            axis=mybir.AxisListType.X, op=A.add)
            V.tensor_reduce(out=OUTSB[:, 1:2], in_=CNT_C,
                            axis=mybir.AxisListType.X, op=A.add)
            nc.sync.dma_start(out=out[:, :], in_=OUTSB)

    return _patch_bass(nc)


def _prep_inputs(embeddings, labels):
    emb = np.ascontiguousarray(np.asarray(embeddings, dtype=np.float32)).reshape(B, O * H)
    posf = (np.asarray(labels) == 1).astype(np.float32)  # [B, O]
    in_maps = []
    for c in range(N_CORES):
        sl = slice(c * B_SHARD, (c + 1) * B_SHARD)
        pm = posf[sl].reshape(N_TILES, P, O).transpose(1, 2, 0).reshape(P, O * N_TILES)
        in_maps.append(
            {
                "emb": np.ascontiguousarray(emb[sl]),
                "pmask": np.ascontiguousarray(pm),
            }
        )
    return in_maps


def kernel(embeddings, labels):
    from concourse.bass_utils import run_bass_kernel_spmd

    if "nc" not in _CACHE:
        _CACHE["nc"] = build()
    nc = _CACHE["nc"]

    in_maps = _prep_inputs(embeddings, labels)
    res = run_bass_kernel_spmd(nc, in_maps, core_ids=list(range(N_CORES)))
    loss_sum = 0.0
    count_sum = 0.0
    for r in res.results:
        o = r["out"].astype(np.float64)
        loss_sum += o[:, 0].sum()
        count_sum += o[:, 1].sum()
    return np.float32(loss_sum / max(count_sum, 1.0))
